# revision 35
# baseline (speedup 1.0000x reference)
"""Trainium2 Bass kernel: memory-slot cross-attention (nn_LocalConstructorMulti).

Reference computation (per batch b):
    Q  = memory_slots @ Wq.T                      [slots, BD]    (shared over b)
    K  = hs_b @ Wk.T                              [S, BD]
    V  = hs_b @ Wv.T                              [S, BD]
    s  = (Q_h . K_h) / sqrt(HD)  + mask           [heads, slots, S]
    p  = softmax(s, axis=S)
    o  = p @ V_h                                  [heads, slots, HD]
    y  = concat_h(o) @ Wo.T                       [slots, HID]

Sharding: 8 cores = 4 batches x 2 head-groups (4 heads / 256 bottleneck dims
each).  Each core sees the full hidden states of its batch and a 256-wide
slice of Wk/Wv/Wo, computes the full softmax locally over its heads, and
produces a partial y (contribution of its 4 heads).  The host sums the two
partials per batch -- o_proj is linear in the per-head outputs.

Key layout/engine decisions (rebuilt from the TimelineSim bottleneck
analysis of v1: PE sequencer + HWDGE were saturated by 2434 small matmuls and
519 small-descriptor DMAs while the PE array itself was 45% idle; v1 sim
413.7us -> this version 168.1us):

  - hs is shipped ONCE, as bf16, host-preshuffled into [8 blocks, 128 ki,
    16 pair, 2 two, 512 col] and streamed in 2-pair chunks with 1-4 KiB
    contiguous per-partition lines; block 0 is processed as two 256-col
    halves so the first V chain starts after ~a quarter block has landed.
  - The fp8 copy feeding the K-projection is derived ON-DEVICE by casting
    copies alternating over the otherwise-idle DVE/ACT engines (saves 16 MB
    of DMA on the sequential ~332 GB/s HBM stream, the co-bottleneck).
  - K-projection runs fp8 MatmulPerfMode.DoubleRow: stationary wk8[128,2,128]
    x moving hs8[128,2,512] contracts TWO 128-deep k-subtiles per
    instruction.  fp8 noise on the K side is attenuated ~64x by the
    near-uniform softmax (~0.1% contribution).  All fp8 range scaling lives
    in Wk (x512 lifts N(0,1/4096) entries into e4m3 normal range); the
    1/512 and 1/sqrt(HD) factors are folded into the host-computed Q.
  - V-projection stays bf16 (V noise passes straight into the output, and
    real-HW DoubleRow is only ~1.44x so an fp8 residual-compensated V would
    be slower): stationary hs blocks [128,128] x moving wv [128,256],
    PSUM-accumulated over all 32 k-subtiles at full 128x128x256 occupancy.
  - Attention is emitted inline per block: scores per 128-row tile with TWO
    head-pairs stacked on partitions (stationary kt[128,128rows] x moving
    block-diagonal q2[128,16]); the mask is a per-partition bias fused into
    the Exp activation; o = p^T @ V_aug is ONE 32-instruction PSUM chain
    computing all 4 heads at once (stationary pt padded to 32-partition
    head stride for the engine partition-offset rule; the ones-column gives
    sum(p) for free and normalization reads PSUM directly).
  - o_proj is emitted transposed: yT[8, 4096] = ot[bd,8]^T @ woT[bd,4096] in
    16 mov-512 matmuls, drains alternating ACT/DVE, output DMA'd per 2 segs;
    the host adds the two head-group partials per batch.
  - Q (8x512, 0.02% of FLOPs) is computed on host and shipped pre-scaled as
    the block-diagonal q2 operand; every PSUM accumulator owns a full bank
    (PE-W + DVE/ACT-R same-bank erratum); wo/q2/mb DMAs are scheduled out of
    the hs stream so they cause no PE bubble.
"""

import sys

if "/opt/trn_rl_repo" not in sys.path:
    sys.path.insert(0, "/opt/trn_rl_repo")

import ml_dtypes
import numpy as np

import concourse.bass as bass  # noqa: F401  (AP helpers)
import concourse.mybir as mybir
import concourse.tile as tile
from concourse import bacc
from concourse.bass_utils import run_bass_kernel_spmd
from concourse.masks import make_identity

BF16 = mybir.dt.bfloat16
FP8 = mybir.dt.float8e4
F32 = mybir.dt.float32
npbf16 = ml_dtypes.bfloat16
npfp8 = ml_dtypes.float8_e4m3

B, S, HID = 4, 4096, 4096
SLOTS, HEADS, BD = 8, 8, 512
HD = BD // HEADS  # 64
N_CORES = 8
GROUPS = N_CORES // B  # head-groups per batch
HPC = HEADS // GROUPS  # heads per core = 4
BDC = HPC * HD  # bottleneck slice per core = 256
MASK_NEG = -30000.0

# hs8 is cast on-device from the bf16 copy (unscaled: K-path noise is
# attenuated ~64x by the softmax, so e4m3 subnormal loss is irrelevant);
# all fp8 range scaling lives in Wk (N(0, 1/4096) entries x512 -> N(0, 8))
WK_SCALE = 512.0
# total score scale 1/sqrt(HD) divided back out of the device-side K product
Q_SCALE = 1.0 / (np.sqrt(HD) * WK_SCALE)

NBLK = 8  # column blocks of the sequence
CBLK = S // NBLK  # 512 columns per block
NPAIR = HID // 256  # 16 k-subtile pairs
NRT = S // 128  # 32 row tiles

# test.py can flip this to capture an NTFF profile; harness never touches it.
TRACE = False
TRACE_CORES = None
LAST_RESULT = None

_cache = {}


def _build_module():
    """Emit + compile the single-core Bass module (same NEFF on all cores)."""
    nc = bacc.Bacc("TRN2", target_bir_lowering=False, debug=False, num_devices=N_CORES)

    hsbT = nc.dram_tensor("hsbT", [NBLK, 128, NPAIR, 2, CBLK], BF16, kind="ExternalInput").ap()
    wk8T = nc.dram_tensor("wk8T", [128, NPAIR, 2, BDC], FP8, kind="ExternalInput").ap()
    wvT = nc.dram_tensor("wvT", [128, NPAIR, 2, BDC], BF16, kind="ExternalInput").ap()
    woT = nc.dram_tensor("woT", [128, 2, HID], BF16, kind="ExternalInput").ap()
    q2T = nc.dram_tensor("q2T", [128, 2, 2 * SLOTS], BF16, kind="ExternalInput").ap()
    mbT = nc.dram_tensor("mbT", [128, NRT], F32, kind="ExternalInput").ap()
    ypT = nc.dram_tensor("ypT", [SLOTS, HID], F32, kind="ExternalOutput").ap()

    DR = mybir.MatmulPerfMode.DoubleRow

    with tile.TileContext(nc) as tc:
        with (
            tc.tile_pool(name="consts", bufs=1) as consts,
            tc.tile_pool(name="hs8p", bufs=2) as hs8p,
            tc.tile_pool(name="hsbp", bufs=2) as hsbp,
        ):
            # ---- resident weights / tables -------------------------------
            wk8_sb = consts.tile([128, NPAIR, 2, BDC], FP8)
            wv_sb = consts.tile([128, NPAIR, 2, BDC], BF16)
            wo_sb = consts.tile([128, 2, HID], BF16)
            q2_sb = consts.tile([128, 2, 2 * SLOTS], BF16)
            mb_sb = consts.tile([128, NRT], F32)
            ident = consts.tile([128, 128], BF16)
            make_identity(nc, ident)

            # ---- persistent intermediates --------------------------------
            kt_sb = consts.tile([128, 2, S], BF16)  # K.T x512  [bd, rows]
            v_sb = consts.tile([128, NRT, HPC, HD + 1], BF16)  # V rows + ones
            nc.vector.memset(v_sb[:, :, :, HD : HD + 1], 1.0)
            # exp(scores).T, head stride padded to 32 so each head's o-block
            # lands on a 32-partition boundary (engine partition-offset rule)
            pt_sb = consts.tile([128, NRT, HPC, 32], BF16)
            nc.vector.memset(pt_sb, 0.0)

            # ---- K/V projections + inline attention, one streamed pass ---
            # attention for each sub-block (scores -> exp -> o-chain partial)
            # is emitted right after its projections: the old 15-20us serial
            # attention tail folds into the DMA/PE overlap window.
            def emit_attn(rt0, nrt_sub):
                for i in range(rt0, rt0 + nrt_sub):
                    s_ps = sps.tile([128, 512], F32, tag="s")
                    for m2 in range(2):
                        nc.tensor.matmul(
                            s_ps[:, m2 * 16 : (m2 + 1) * 16],
                            kt_sb[:, m2, i * 128 : (i + 1) * 128],
                            q2_sb[:, m2, :],
                            start=True,
                            stop=True,
                        )
                    nc.scalar.activation(
                        out=pt_sb[:, i, :, 0:SLOTS],
                        in_=s_ps[:, 0 : HPC * SLOTS].rearrange(
                            "p (h n) -> p h n", h=HPC
                        ),
                        func=mybir.ActivationFunctionType.Exp,
                        bias=mb_sb[:, i : i + 1],
                        scale=1.0,
                    )
                for i in range(rt0, rt0 + nrt_sub):
                    nc.tensor.matmul(
                        o_ps[:, 0 : HPC * (HD + 1)],
                        pt_sb[:, i, :, :],
                        v_sb[:, i, :, :],
                        start=(i == 0),
                        stop=(i == NRT - 1),
                    )

            with (
                tc.tile_pool(name="ktps", bufs=2, space="PSUM") as ktps,
                tc.tile_pool(name="vps", bufs=2, space="PSUM") as vps,
                tc.tile_pool(name="sps", bufs=2, space="PSUM") as sps,
                tc.tile_pool(name="ops", bufs=1, space="PSUM") as ops,
            ):
                o_ps = ops.tile([128, 512], F32)
                cast_engines = (
                    nc.vector.tensor_copy,
                    nc.scalar.copy,
                )
                # block 0 is processed as two 256-col halves so the first V
                # chain starts after ~1/4 of a block has streamed instead of
                # a full one (PE fill); the rest are full 512-col blocks
                SUBS = [(0, 0, CBLK // 2), (0, CBLK // 2, CBLK // 2)] + [
                    (b, 0, CBLK) for b in range(1, NBLK)
                ]
                for sub, (blk, c0, w) in enumerate(SUBS):
                    # hs DMAs arrive in pair-group chunks so the K/V chains
                    # start consuming before the whole sub-block has landed
                    hsb_t = hsbp.tile([128, NPAIR, 2, CBLK], BF16, tag="hsb")
                    hs8_t = hs8p.tile([128, NPAIR, 2, CBLK], FP8, tag="hs8")
                    nch = 8 * w // CBLK
                    h = NPAIR // nch
                    for q in range(nch):
                        if sub == 0:
                            # wv pair-chunks ride along with the first hsb
                            # chunks: the V chain consumes pairs in order
                            nc.sync.dma_start(
                                out=wv_sb[:, q * h : (q + 1) * h],
                                in_=wvT[:, q * h : (q + 1) * h],
                            )
                        nc.sync.dma_start(
                            out=hsb_t[:, q * h : (q + 1) * h, :, 0:w],
                            in_=hsbT[blk][:, q * h : (q + 1) * h, :, c0 : c0 + w],
                        )
                        # hs8 is derived on-device: a casting copy on the
                        # otherwise-idle DVE/ACT engines replaces 16 MB of
                        # fp8 DMA traffic
                        cast_engines[q % 2](
                            out=hs8_t[:, q * h : (q + 1) * h, :, 0:w],
                            in_=hsb_t[:, q * h : (q + 1) * h, :, 0:w],
                        )
                    if sub == 0:
                        nc.sync.dma_start(out=wk8_sb, in_=wk8T)
                        nc.sync.dma_start(out=q2_sb, in_=q2T)
                        nc.sync.dma_start(out=mb_sb, in_=mbT)
                    if sub == len(SUBS) - 1:
                        # wo is first needed by o_proj at the very end; keep
                        # it out of the hs stream so it causes no PE bubble
                        nc.sync.dma_start(out=wo_sb, in_=woT)
                    # V chains: bf16, stationary hs blocks, full-bank psum
                    for r in range(w // 128):
                        v_ps = vps.tile([128, 512], F32, tag="v")
                        for p in range(NPAIR):
                            for t in range(2):
                                nc.tensor.matmul(
                                    v_ps[:, 0:BDC],
                                    hsb_t[:, p, t, r * 128 : (r + 1) * 128],
                                    wv_sb[:, p, t, :],
                                    start=(p == 0 and t == 0),
                                    stop=(p == NPAIR - 1 and t == 1),
                                )
                        rt = (blk * CBLK + c0) // 128 + r
                        nc.vector.tensor_copy(
                            out=v_sb[:, rt, :, 0:HD],
                            in_=v_ps[:, 0:BDC].rearrange("p (h d) -> p h d", h=HPC),
                        )
                    # KT chains: fp8 DoubleRow, 2 k-subtiles per instruction
                    for m2 in range(2):
                        kt_ps = ktps.tile([128, CBLK], F32, tag="kt")
                        for p in range(NPAIR):
                            nc.tensor.matmul(
                                kt_ps[:, 0:w],
                                wk8_sb[:, p, :, m2 * 128 : (m2 + 1) * 128],
                                hs8_t[:, p, :, 0:w],
                                start=(p == 0),
                                stop=(p == NPAIR - 1),
                                perf_mode=DR,
                            )
                        nc.scalar.copy(
                            out=kt_sb[
                                :, m2, blk * CBLK + c0 : blk * CBLK + c0 + w
                            ],
                            in_=kt_ps[:, 0:w],
                        )
                    emit_attn((blk * CBLK + c0) // 128, w // 128)
                # normalize straight out of PSUM: o / sum(p) via the ones
                # column (the o-chain has stopped, so no same-bank PE-W
                # overlaps these reads)
                recip = consts.tile([SLOTS, HPC], F32)
                o_slot = consts.tile([SLOTS, BDC], BF16)
                for h in range(HPC):
                    nc.vector.reciprocal(
                        out=recip[:, h : h + 1],
                        in_=o_ps[h * 32 : h * 32 + SLOTS,
                                 h * (HD + 1) + HD : (h + 1) * (HD + 1)],
                    )
                for h in range(HPC):
                    nc.vector.tensor_scalar_mul(
                        out=o_slot[:, h * HD : (h + 1) * HD],
                        in0=o_ps[h * 32 : h * 32 + SLOTS,
                                 h * (HD + 1) : h * (HD + 1) + HD],
                        scalar1=recip[:, h : h + 1],
                    )

            # ---- transpose o to [bd, slots] ------------------------------
            ot_sb = consts.tile([128, 2, SLOTS], BF16)
            with tc.tile_pool(name="tps", bufs=2, space="PSUM") as tps:
                for j in range(2):
                    t_ps = tps.tile([128, 1024], BF16, tag="t")
                    nc.tensor.transpose(
                        t_ps[:, 0:SLOTS],
                        o_slot[:, j * 128 : (j + 1) * 128],
                        ident[:SLOTS, :SLOTS],
                    )
                    nc.scalar.copy(out=ot_sb[:, j, :], in_=t_ps[:, 0:SLOTS])

            # ---- partial o_proj, transposed: yT = ot^T @ woT -------------
            # drains alternate ACT/DVE so neither engine paces the phase;
            # each 512-seg is DMA'd out as soon as its copy lands
            y_sb = consts.tile([SLOTS, HID], F32)
            with tc.tile_pool(name="yps", bufs=4, space="PSUM") as yps:
                for seg in range(HID // 512):
                    y_ps = yps.tile([SLOTS, 512], F32, tag="y")
                    for j in range(2):
                        nc.tensor.matmul(
                            y_ps,
                            ot_sb[:, j, :],
                            wo_sb[:, j, seg * 512 : (seg + 1) * 512],
                            start=(j == 0),
                            stop=(j == 1),
                        )
                    if seg % 2 == 0:
                        nc.scalar.copy(
                            out=y_sb[:, seg * 512 : (seg + 1) * 512], in_=y_ps
                        )
                    else:
                        nc.vector.tensor_copy(
                            out=y_sb[:, seg * 512 : (seg + 1) * 512], in_=y_ps
                        )
                    if seg % 2 == 1:
                        nc.sync.dma_start(
                            out=ypT[:, (seg - 1) * 512 : (seg + 1) * 512],
                            in_=y_sb[:, (seg - 1) * 512 : (seg + 1) * 512],
                        )

    nc.compile()
    return nc


def _get_module():
    if "m" not in _cache:
        _cache["m"] = _build_module()
    return _cache["m"]


def _shuffle_hs(hsT_np, dtype, scale=1.0):
    """[HID, S] -> [NBLK, 128, NPAIR, 2, CBLK] with the (pair, two, ki)
    k-decomposition on axis 0 and (blk, col) on axis 1."""
    a = hsT_np.reshape(NPAIR, 2, 128, NBLK, CBLK)
    a = a.transpose(3, 2, 0, 1, 4)  # blk, ki, pair, two, col
    if scale != 1.0:
        a = a * np.float32(scale)
    return np.ascontiguousarray(a.astype(dtype))


def _prep_in_maps(hs, mask, ms, Wq, Wk, Wv, Wo):
    """Shard the full inputs into 8 per-core input maps (host-side)."""
    WkT = Wk.T.astype(np.float32)  # [HID, BD]
    WvT = Wv.T.astype(np.float32)
    WoT = Wo.T.astype(np.float32)  # [BD, HID]
    Q = (ms @ Wq.T).astype(np.float32)  # [SLOTS, BD]

    hsb = []
    mb = []
    for b in range(B):
        hsT = np.ascontiguousarray(hs[b].T)  # [HID, S]
        hsb.append(_shuffle_hs(hsT, npbf16))
        mb.append(
            np.ascontiguousarray(
                np.where(mask[b] == 0, np.float32(MASK_NEG), np.float32(0.0))
                .astype(np.float32)
                .reshape(NRT, 128)
                .T
            )
        )

    in_maps = []
    for c in range(N_CORES):
        b, g = c // GROUPS, c % GROUPS
        sl = slice(g * BDC, (g + 1) * BDC)
        wk8 = (
            (WkT[:, sl] * np.float32(WK_SCALE))
            .reshape(NPAIR, 2, 128, BDC)
            .transpose(2, 0, 1, 3)
        )
        wv = WvT[:, sl].reshape(NPAIR, 2, 128, BDC).transpose(2, 0, 1, 3)
        wo = WoT[sl].reshape(2, 128, HID).transpose(1, 0, 2)
        q2 = np.zeros((128, 2, 2 * SLOTS), np.float32)
        for m2 in range(2):
            h0 = g * HPC + 2 * m2
            q2[0:64, m2, 0:SLOTS] = Q[:, h0 * HD : (h0 + 1) * HD].T * Q_SCALE
            q2[64:128, m2, SLOTS : 2 * SLOTS] = (
                Q[:, (h0 + 1) * HD : (h0 + 2) * HD].T * Q_SCALE
            )
        in_maps.append(
            {
                "hsbT": hsb[b],
                "wk8T": np.ascontiguousarray(wk8.astype(npfp8)),
                "wvT": np.ascontiguousarray(wv.astype(npbf16)),
                "woT": np.ascontiguousarray(wo.astype(npbf16)),
                "q2T": np.ascontiguousarray(q2.astype(npbf16)),
                "mbT": mb[b],
            }
        )
    return in_maps


def kernel(hidden_states, attention_mask, memory_slots, Wq, Wk, Wv, Wo):
    global LAST_RESULT
    hs = np.asarray(hidden_states, dtype=np.float32)
    mask = np.asarray(attention_mask)
    ms = np.asarray(memory_slots, dtype=np.float32)
    Wq = np.asarray(Wq, dtype=np.float32)
    Wk = np.asarray(Wk, dtype=np.float32)
    Wv = np.asarray(Wv, dtype=np.float32)
    Wo = np.asarray(Wo, dtype=np.float32)

    nc = _get_module()
    in_maps = _prep_in_maps(hs, mask, ms, Wq, Wk, Wv, Wo)

    kwargs = {}
    if TRACE:
        kwargs = {"trace": True}
        if TRACE_CORES is not None:
            kwargs["trace_cores"] = TRACE_CORES
    res = run_bass_kernel_spmd(nc, in_maps, core_ids=list(range(N_CORES)), **kwargs)
    LAST_RESULT = res

    yp = [r["ypT"] for r in res.results]  # each [SLOTS, HID] f32
    y = np.stack(
        [yp[GROUPS * b] + yp[GROUPS * b + 1] for b in range(B)], axis=0
    )
    return np.ascontiguousarray(y.astype(np.float32))


# revision 39
# speedup vs baseline: 1.1355x; 1.1355x over previous
"""Trainium2 Bass kernel: memory-slot cross-attention (nn_LocalConstructorMulti).

Reference computation (per batch b):
    Q  = memory_slots @ Wq.T                      [slots, BD]    (shared over b)
    K  = hs_b @ Wk.T                              [S, BD]
    V  = hs_b @ Wv.T                              [S, BD]
    s  = (Q_h . K_h) / sqrt(HD)  + mask           [heads, slots, S]
    p  = softmax(s, axis=S)
    o  = p @ V_h                                  [heads, slots, HD]
    y  = concat_h(o) @ Wo.T                       [slots, HID]

Sharding: 8 cores = 4 batches x 2 head-groups (4 heads / 256 bottleneck dims
each).  Each core sees the full hidden states of its batch and a 256-wide
slice of Wk/Wv/Wo, computes the full softmax locally over its heads, and
produces a partial y (contribution of its 4 heads).  The host sums the two
partials per batch -- o_proj is linear in the per-head outputs.

Key layout/engine decisions (rebuilt from the TimelineSim bottleneck
analysis of v1: PE sequencer + HWDGE were saturated by 2434 small matmuls and
519 small-descriptor DMAs while the PE array itself was 45% idle; v1 sim
413.7us -> this version 168.1us):

  - hs is shipped ONCE, as bf16, host-preshuffled into [8 blocks, 128 ki,
    16 pair, 2 two, 512 col] and streamed in 2-pair chunks with 1-4 KiB
    contiguous per-partition lines; block 0 is processed as two 256-col
    halves so the first V chain starts after ~a quarter block has landed.
  - The fp8 copy feeding the K-projection is derived ON-DEVICE by casting
    copies alternating over the otherwise-idle DVE/ACT engines (saves 16 MB
    of DMA on the sequential ~332 GB/s HBM stream, the co-bottleneck).
  - K-projection runs fp8 MatmulPerfMode.DoubleRow: stationary wk8[128,2,128]
    x moving hs8[128,2,512] contracts TWO 128-deep k-subtiles per
    instruction.  fp8 noise on the K side is attenuated ~64x by the
    near-uniform softmax (~0.1% contribution).  All fp8 range scaling lives
    in Wk (x512 lifts N(0,1/4096) entries into e4m3 normal range); the
    1/512 and 1/sqrt(HD) factors are folded into the host-computed Q.
  - V-projection stays bf16 (V noise passes straight into the output, and
    real-HW DoubleRow is only ~1.44x so an fp8 residual-compensated V would
    be slower): stationary hs blocks [128,128] x moving wv [128,256],
    PSUM-accumulated over all 32 k-subtiles at full 128x128x256 occupancy.
  - Attention is emitted inline per block: scores per 128-row tile with TWO
    head-pairs stacked on partitions (stationary kt[128,128rows] x moving
    block-diagonal q2[128,16]); the mask is a per-partition bias fused into
    the Exp activation; o = p^T @ V_aug is ONE 32-instruction PSUM chain
    computing all 4 heads at once (stationary pt padded to 32-partition
    head stride for the engine partition-offset rule; the ones-column gives
    sum(p) for free and normalization reads PSUM directly).
  - o_proj is emitted transposed: yT[8, 4096] = ot[bd,8]^T @ woT[bd,4096] in
    16 mov-512 matmuls, drains alternating ACT/DVE, output DMA'd per 2 segs;
    the host adds the two head-group partials per batch.
  - Q (8x512, 0.02% of FLOPs) is computed on host and shipped pre-scaled as
    the block-diagonal q2 operand; every PSUM accumulator owns a full bank
    (PE-W + DVE/ACT-R same-bank erratum); wo/q2/mb DMAs are scheduled out of
    the hs stream so they cause no PE bubble.
"""

import sys

if "/opt/trn_rl_repo" not in sys.path:
    sys.path.insert(0, "/opt/trn_rl_repo")

import ml_dtypes
import numpy as np

import concourse.bass as bass  # noqa: F401  (AP helpers)
import concourse.mybir as mybir
import concourse.tile as tile
from concourse import bacc
from concourse.bass_utils import run_bass_kernel_spmd
from concourse.masks import make_identity

BF16 = mybir.dt.bfloat16
FP8 = mybir.dt.float8e4
F32 = mybir.dt.float32
npbf16 = ml_dtypes.bfloat16
npfp8 = ml_dtypes.float8_e4m3

B, S, HID = 4, 4096, 4096
SLOTS, HEADS, BD = 8, 8, 512
HD = BD // HEADS  # 64
N_CORES = 8
GROUPS = N_CORES // B  # head-groups per batch
HPC = HEADS // GROUPS  # heads per core = 4
BDC = HPC * HD  # bottleneck slice per core = 256
MASK_NEG = -30000.0

NBLK = 8  # column blocks of the sequence
CBLK = S // NBLK  # 512 columns per block
NPAIR = HID // 256  # 16 k-subtile pairs
NRT = S // 128  # 32 row tiles

# test.py can flip this to capture an NTFF profile; harness never touches it.
TRACE = False
TRACE_CORES = None
LAST_RESULT = None

_cache = {}


def _build_module():
    """Emit + compile the single-core Bass module (same NEFF on all cores)."""
    nc = bacc.Bacc("TRN2", target_bir_lowering=False, debug=False, num_devices=N_CORES)

    hsbT = nc.dram_tensor("hsbT", [NBLK, 128, NPAIR, 2, CBLK], BF16, kind="ExternalInput").ap()
    wvT = nc.dram_tensor("wvT", [128, NPAIR, 2, BDC], BF16, kind="ExternalInput").ap()
    woT = nc.dram_tensor("woT", [128, 2, HID], BF16, kind="ExternalInput").ap()
    qkT = nc.dram_tensor("qkT", [128, NPAIR, 2, HPC * SLOTS], BF16, kind="ExternalInput").ap()
    mbT = nc.dram_tensor("mbT", [128, NRT], F32, kind="ExternalInput").ap()
    ypT = nc.dram_tensor("ypT", [SLOTS, HID], F32, kind="ExternalOutput").ap()

    with tile.TileContext(nc) as tc:
        with (
            tc.tile_pool(name="consts", bufs=1) as consts,
            tc.tile_pool(name="hsbp", bufs=2) as hsbp,
        ):
            # ---- resident weights / tables -------------------------------
            wv_sb = consts.tile([128, NPAIR, 2, BDC], BF16)
            wo_sb = consts.tile([128, 2, HID], BF16)
            qk_sb = consts.tile([128, NPAIR, 2, HPC * SLOTS], BF16)
            mb_sb = consts.tile([128, NRT], F32)
            ident = consts.tile([128, 128], BF16)
            make_identity(nc, ident)

            # ---- persistent intermediates --------------------------------
            v_sb = consts.tile([128, NRT, HPC, HD + 1], BF16)  # V rows + ones
            nc.vector.memset(v_sb[:, :, :, HD : HD + 1], 1.0)
            # exp(scores).T, head stride padded to 32 so each head's o-block
            # lands on a 32-partition boundary (engine partition-offset rule)
            pt_sb = consts.tile([128, NRT, HPC, 32], BF16)
            nc.vector.memset(pt_sb, 0.0)

            # ---- V projection + Q-folded scores + inline attention -------
            # There is no K projection: scores = Q.K = (Q @ Wk) . hs, and
            # qk = (Q @ Wk)/sqrt(HD) is a tiny [32, HID] host-computed
            # operand.  Each hs stationary block therefore serves TWO
            # matmuls: the V chain (moving wv, 256 wide) and the score
            # chain (moving qk, 32 wide) -- 8x fewer K-side MACs and zero
            # extra weight loads.
            with (
                tc.tile_pool(name="vps", bufs=2, space="PSUM") as vps,
                tc.tile_pool(name="sps", bufs=2, space="PSUM") as sps,
                tc.tile_pool(name="ops", bufs=1, space="PSUM") as ops,
            ):
                o_ps = ops.tile([128, 512], F32)
                # block 0 is processed as two 256-col halves so the first V
                # chain starts after ~1/4 of a block has streamed instead of
                # a full one (PE fill); the rest are full 512-col blocks
                SUBS = [(0, 0, CBLK // 2), (0, CBLK // 2, CBLK // 2)] + [
                    (b, 0, CBLK) for b in range(1, NBLK)
                ]
                for sub, (blk, c0, w) in enumerate(SUBS):
                    # hs DMAs arrive in pair-group chunks so the V/score
                    # chains start consuming before the whole sub-block lands
                    hsb_t = hsbp.tile([128, NPAIR, 2, CBLK], BF16, tag="hsb")
                    nch = 8 * w // CBLK
                    h = NPAIR // nch
                    for q in range(nch):
                        if sub == 0:
                            # wv pair-chunks ride along with the first hsb
                            # chunks: the V chain consumes pairs in order
                            nc.sync.dma_start(
                                out=wv_sb[:, q * h : (q + 1) * h],
                                in_=wvT[:, q * h : (q + 1) * h],
                            )
                        nc.sync.dma_start(
                            out=hsb_t[:, q * h : (q + 1) * h, :, 0:w],
                            in_=hsbT[blk][:, q * h : (q + 1) * h, :, c0 : c0 + w],
                        )
                    if sub == 0:
                        nc.sync.dma_start(out=qk_sb, in_=qkT)
                        nc.sync.dma_start(out=mb_sb, in_=mbT)
                    if sub == len(SUBS) - 1:
                        # wo is first needed by o_proj at the very end; keep
                        # it out of the hs stream so it causes no PE bubble
                        nc.sync.dma_start(out=wo_sb, in_=woT)
                    # V + score chains per 128-row subtile, sharing the
                    # stationary hs block; both accumulate over all 32
                    # k-subtiles
                    for r in range(w // 128):
                        rt = (blk * CBLK + c0) // 128 + r
                        v_ps = vps.tile([128, 512], F32, tag="v")
                        s_ps = sps.tile([128, 512], F32, tag="s")
                        for p in range(NPAIR):
                            for t in range(2):
                                st = p == 0 and t == 0
                                sp = p == NPAIR - 1 and t == 1
                                stat = hsb_t[:, p, t, r * 128 : (r + 1) * 128]
                                nc.tensor.matmul(
                                    v_ps[:, 0:BDC],
                                    stat,
                                    wv_sb[:, p, t, :],
                                    start=st,
                                    stop=sp,
                                )
                                nc.tensor.matmul(
                                    s_ps[:, 0 : HPC * SLOTS],
                                    stat,
                                    qk_sb[:, p, t, :],
                                    start=st,
                                    stop=sp,
                                )
                        nc.vector.tensor_copy(
                            out=v_sb[:, rt, :, 0:HD],
                            in_=v_ps[:, 0:BDC].rearrange(
                                "p (h d) -> p h d", h=HPC
                            ),
                        )
                        nc.scalar.activation(
                            out=pt_sb[:, rt, :, 0:SLOTS],
                            in_=s_ps[:, 0 : HPC * SLOTS].rearrange(
                                "p (h n) -> p h n", h=HPC
                            ),
                            func=mybir.ActivationFunctionType.Exp,
                            bias=mb_sb[:, rt : rt + 1],
                            scale=1.0,
                        )
                    # o-chain partial for this sub-block's rowtiles
                    rt0 = (blk * CBLK + c0) // 128
                    for i in range(rt0, rt0 + w // 128):
                        nc.tensor.matmul(
                            o_ps[:, 0 : HPC * (HD + 1)],
                            pt_sb[:, i, :, :],
                            v_sb[:, i, :, :],
                            start=(i == 0),
                            stop=(i == NRT - 1),
                        )
                # normalize straight out of PSUM: o / sum(p) via the ones
                # column (the o-chain has stopped, so no same-bank PE-W
                # overlaps these reads)
                recip = consts.tile([SLOTS, HPC], F32)
                o_slot = consts.tile([SLOTS, BDC], BF16)
                for h in range(HPC):
                    nc.vector.reciprocal(
                        out=recip[:, h : h + 1],
                        in_=o_ps[h * 32 : h * 32 + SLOTS,
                                 h * (HD + 1) + HD : (h + 1) * (HD + 1)],
                    )
                for h in range(HPC):
                    nc.vector.tensor_scalar_mul(
                        out=o_slot[:, h * HD : (h + 1) * HD],
                        in0=o_ps[h * 32 : h * 32 + SLOTS,
                                 h * (HD + 1) : h * (HD + 1) + HD],
                        scalar1=recip[:, h : h + 1],
                    )

            # ---- transpose o to [bd, slots] ------------------------------
            ot_sb = consts.tile([128, 2, SLOTS], BF16)
            with tc.tile_pool(name="tps", bufs=2, space="PSUM") as tps:
                for j in range(2):
                    t_ps = tps.tile([128, 1024], BF16, tag="t")
                    nc.tensor.transpose(
                        t_ps[:, 0:SLOTS],
                        o_slot[:, j * 128 : (j + 1) * 128],
                        ident[:SLOTS, :SLOTS],
                    )
                    nc.scalar.copy(out=ot_sb[:, j, :], in_=t_ps[:, 0:SLOTS])

            # ---- partial o_proj, transposed: yT = ot^T @ woT -------------
            # drains alternate ACT/DVE so neither engine paces the phase;
            # each 512-seg is DMA'd out as soon as its copy lands
            y_sb = consts.tile([SLOTS, HID], F32)
            with tc.tile_pool(name="yps", bufs=4, space="PSUM") as yps:
                for seg in range(HID // 512):
                    y_ps = yps.tile([SLOTS, 512], F32, tag="y")
                    for j in range(2):
                        nc.tensor.matmul(
                            y_ps,
                            ot_sb[:, j, :],
                            wo_sb[:, j, seg * 512 : (seg + 1) * 512],
                            start=(j == 0),
                            stop=(j == 1),
                        )
                    if seg % 2 == 0:
                        nc.scalar.copy(
                            out=y_sb[:, seg * 512 : (seg + 1) * 512], in_=y_ps
                        )
                    else:
                        nc.vector.tensor_copy(
                            out=y_sb[:, seg * 512 : (seg + 1) * 512], in_=y_ps
                        )
                    if seg % 2 == 1:
                        nc.sync.dma_start(
                            out=ypT[:, (seg - 1) * 512 : (seg + 1) * 512],
                            in_=y_sb[:, (seg - 1) * 512 : (seg + 1) * 512],
                        )

    nc.compile()
    return nc


def _get_module():
    if "m" not in _cache:
        _cache["m"] = _build_module()
    return _cache["m"]


def _shuffle_hs(hsT_np, dtype, scale=1.0):
    """[HID, S] -> [NBLK, 128, NPAIR, 2, CBLK] with the (pair, two, ki)
    k-decomposition on axis 0 and (blk, col) on axis 1."""
    a = hsT_np.reshape(NPAIR, 2, 128, NBLK, CBLK)
    a = a.transpose(3, 2, 0, 1, 4)  # blk, ki, pair, two, col
    if scale != 1.0:
        a = a * np.float32(scale)
    return np.ascontiguousarray(a.astype(dtype))


def _prep_in_maps(hs, mask, ms, Wq, Wk, Wv, Wo):
    """Shard the full inputs into 8 per-core input maps (host-side)."""
    WkT = Wk.T.astype(np.float32)  # [HID, BD]
    WvT = Wv.T.astype(np.float32)
    WoT = Wo.T.astype(np.float32)  # [BD, HID]
    Q = (ms @ Wq.T).astype(np.float32)  # [SLOTS, BD]

    hsb = []
    mb = []
    for b in range(B):
        hsT = np.ascontiguousarray(hs[b].T)  # [HID, S]
        hsb.append(_shuffle_hs(hsT, npbf16))
        mb.append(
            np.ascontiguousarray(
                np.where(mask[b] == 0, np.float32(MASK_NEG), np.float32(0.0))
                .astype(np.float32)
                .reshape(NRT, 128)
                .T
            )
        )

    # Q folded into Wk: qk[h, n, :] = (Q_h[n, :] @ Wk_h) / sqrt(HD), so
    # scores = qk . hs directly -- the K projection never materializes
    qk = np.empty((HEADS, SLOTS, HID), np.float32)
    for hh in range(HEADS):
        qk[hh] = (Q[:, hh * HD : (hh + 1) * HD] @ WkT[:, hh * HD : (hh + 1) * HD].T
                  ) * np.float32(1.0 / np.sqrt(HD))

    in_maps = []
    for c in range(N_CORES):
        b, g = c // GROUPS, c % GROUPS
        sl = slice(g * BDC, (g + 1) * BDC)
        wv = WvT[:, sl].reshape(NPAIR, 2, 128, BDC).transpose(2, 0, 1, 3)
        wo = WoT[sl].reshape(2, 128, HID).transpose(1, 0, 2)
        # [HID, HPC*SLOTS] -> [128 ki, NPAIR, 2, 32], (h, n) h-major
        qkc = (
            qk[g * HPC : (g + 1) * HPC]
            .reshape(HPC * SLOTS, HID)
            .T.reshape(NPAIR, 2, 128, HPC * SLOTS)
            .transpose(2, 0, 1, 3)
        )
        in_maps.append(
            {
                "hsbT": hsb[b],
                "wvT": np.ascontiguousarray(wv.astype(npbf16)),
                "woT": np.ascontiguousarray(wo.astype(npbf16)),
                "qkT": np.ascontiguousarray(qkc.astype(npbf16)),
                "mbT": mb[b],
            }
        )
    return in_maps


def kernel(hidden_states, attention_mask, memory_slots, Wq, Wk, Wv, Wo):
    global LAST_RESULT
    hs = np.asarray(hidden_states, dtype=np.float32)
    mask = np.asarray(attention_mask)
    ms = np.asarray(memory_slots, dtype=np.float32)
    Wq = np.asarray(Wq, dtype=np.float32)
    Wk = np.asarray(Wk, dtype=np.float32)
    Wv = np.asarray(Wv, dtype=np.float32)
    Wo = np.asarray(Wo, dtype=np.float32)

    nc = _get_module()
    in_maps = _prep_in_maps(hs, mask, ms, Wq, Wk, Wv, Wo)

    kwargs = {}
    if TRACE:
        kwargs = {"trace": True}
        if TRACE_CORES is not None:
            kwargs["trace_cores"] = TRACE_CORES
    res = run_bass_kernel_spmd(nc, in_maps, core_ids=list(range(N_CORES)), **kwargs)
    LAST_RESULT = res

    yp = [r["ypT"] for r in res.results]  # each [SLOTS, HID] f32
    y = np.stack(
        [yp[GROUPS * b] + yp[GROUPS * b + 1] for b in range(B)], axis=0
    )
    return np.ascontiguousarray(y.astype(np.float32))


# revision 44
# speedup vs baseline: 1.4383x; 1.2666x over previous
"""Trainium2 Bass kernel: memory-slot cross-attention (nn_LocalConstructorMulti).

Reference computation (per batch b):
    Q  = memory_slots @ Wq.T                      [slots, BD]    (shared over b)
    K  = hs_b @ Wk.T                              [S, BD]
    V  = hs_b @ Wv.T                              [S, BD]
    s  = (Q_h . K_h) / sqrt(HD)  + mask           [heads, slots, S]
    p  = softmax(s, axis=S)
    o  = p @ V_h                                  [heads, slots, HD]
    y  = concat_h(o) @ Wo.T                       [slots, HID]

NEITHER PROJECTION IS EVER MATERIALIZED.  Both big GEMMs fold away by
associativity around the tiny slot dimension (8 slots x 8 heads = 64):

    scores = Q.(Wk hs^T) = (Q @ Wk) . hs^T        qk: [64, HID], host, 16 MF
    o_num  = p^T (hs Wv^T) = (p^T hs) Wv^T        u:  [HID, 64] on device

That replaces 2 x 4.3 GMAC/core of K/V projections with 2 x 0.54 GMAC of
skinny contractions against hs -- the kernel becomes DMA-bound at the
~332 GB/s HBM stream (cost-model timeline: v1 413.7us -> 111.5us).

Sharding: 8 cores = 4 batches x 2 sequence-halves.  Each core holds its
2048-row half in BOTH orientations (16 MB hsT for the k-contracted score
pass, 16 MB row-major for the s-contracted u pass -- same 32 MB/core as
shipping hs twice was) and computes, for all 8 heads:

    phase 1: s = qk . hsT_half   (stationary hsT blocks, moving qk [128,64];
             mask fused as per-partition Exp bias -> p, all local)
    phase 2: u = p^T hs_half     (stationary row-major hs blocks, moving p;
             den = 1^T p via a ones-column stationary)
    phase 3: z = u @ Wv^T        (32 mov-512 matmuls on the aggregated u)

The host sums the two halves' linear partials (z, den), normalizes, and
applies the 67 MFLOP o_proj (0.05% of the model's FLOPs).  The exp
nonlinearity is the only thing pinning p to the device between the passes;
everything that touches the 256 MB hs tensor stays on-device.

Layout notes: both hs orientations are host-preshuffled into
[blocks, 128 part, subtiles, 512] so every DMA has multi-KiB contiguous
per-partition lines and chains start on pair-group chunk arrival; every
PSUM accumulator owns a full bank or shares one only with other PE-W
accumulation groups (PE-W + DVE/ACT-R same-bank erratum); the score pass
streams first so exp'd p tiles are ready exactly when the u pass needs
them; wv/qk/mb ride outside the hs stream.
"""

import sys

if "/opt/trn_rl_repo" not in sys.path:
    sys.path.insert(0, "/opt/trn_rl_repo")

import ml_dtypes
import numpy as np

import concourse.bass as bass  # noqa: F401  (AP helpers)
import concourse.mybir as mybir
import concourse.tile as tile
from concourse import bacc
from concourse.bass_utils import run_bass_kernel_spmd

BF16 = mybir.dt.bfloat16
F32 = mybir.dt.float32
npbf16 = ml_dtypes.bfloat16

B, S, HID = 4, 4096, 4096
SLOTS, HEADS, BD = 8, 8, 512
HD = BD // HEADS  # 64
N_CORES = 8
HALVES = N_CORES // B  # sequence halves per batch
SL = S // HALVES  # 2048 local rows
HSL = HEADS * SLOTS  # 64 head-slot columns
MASK_NEG = -30000.0

NKS = HID // 128  # 32 k-subtiles
NRT = SL // 128  # 16 local row tiles
TBLK = 4  # hsT column blocks (512 rows each)
RBLK = 8  # row-major k blocks (512 k-cols each)

# test.py can flip this to capture an NTFF profile; harness never touches it.
TRACE = False
TRACE_CORES = None
LAST_RESULT = None

_cache = {}


def _build_module():
    """Emit + compile the single-core Bass module (same NEFF on all cores)."""
    nc = bacc.Bacc("TRN2", target_bir_lowering=False, debug=False, num_devices=N_CORES)

    hstT = nc.dram_tensor("hstT", [TBLK, 128, NKS, SL // TBLK], BF16, kind="ExternalInput").ap()
    hsrT = nc.dram_tensor("hsrT", [RBLK, 128, NRT, HID // RBLK], BF16, kind="ExternalInput").ap()
    qkT = nc.dram_tensor("qkT", [128, NKS, HSL], BF16, kind="ExternalInput").ap()
    wvT = nc.dram_tensor("wvT", [128, NKS, BD], BF16, kind="ExternalInput").ap()
    mbT = nc.dram_tensor("mbT", [128, NRT], F32, kind="ExternalInput").ap()
    zT = nc.dram_tensor("zT", [HSL, BD], F32, kind="ExternalOutput").ap()
    denT = nc.dram_tensor("denT", [1, HSL], F32, kind="ExternalOutput").ap()

    CT = SL // TBLK  # 512 score columns per hsT block
    CR = HID // RBLK  # 512 k columns per row-major block

    with tile.TileContext(nc) as tc:
        with (
            tc.tile_pool(name="consts", bufs=1) as consts,
            tc.tile_pool(name="hstp", bufs=2) as hstp,
            tc.tile_pool(name="hsrp", bufs=2) as hsrp,
        ):
            # ---- resident operands ---------------------------------------
            qk_sb = consts.tile([128, NKS, HSL], BF16)
            wv_sb = consts.tile([128, NKS, BD], BF16)
            mb_sb = consts.tile([128, NRT], F32)
            ones_sb = consts.tile([128, 1], BF16)
            nc.vector.memset(ones_sb, 1.0)

            # ---- persistent intermediates --------------------------------
            pt_sb = consts.tile([128, NRT, HSL], BF16)  # exp(scores)
            u_sb = consts.tile([128, NKS, HSL], BF16)  # u = p^T hs
            den_sb = consts.tile([1, HSL], F32)
            z_sb = consts.tile([HSL, BD], F32)

            with (
                tc.tile_pool(name="sps", bufs=2, space="PSUM") as sps,
                tc.tile_pool(name="ups", bufs=2, space="PSUM") as ups,
                tc.tile_pool(name="dps", bufs=1, space="PSUM") as dps,
                tc.tile_pool(name="zps", bufs=1, space="PSUM") as zps,
            ):
                # ---- phase 1: scores + exp from the k-partitioned half ---
                for blk in range(TBLK):
                    hst_t = hstp.tile([128, NKS, CT], BF16, tag="hst")
                    for q in range(8):
                        h = NKS // 8
                        if blk == 0 and q < 4:
                            # qk/mb ride with the first chunks; they are
                            # tiny and needed by the first chains
                            if q == 0:
                                nc.sync.dma_start(out=qk_sb, in_=qkT)
                                nc.sync.dma_start(out=mb_sb, in_=mbT)
                        nc.sync.dma_start(
                            out=hst_t[:, q * h : (q + 1) * h],
                            in_=hstT[blk][:, q * h : (q + 1) * h],
                        )
                    if blk == TBLK - 1:
                        # wv is first needed by phase 3
                        nc.sync.dma_start(out=wv_sb, in_=wvT)
                    for r in range(CT // 128):
                        rt = blk * (CT // 128) + r
                        s_ps = sps.tile([128, 512], F32, tag="s")
                        for k in range(NKS):
                            nc.tensor.matmul(
                                s_ps[:, 0:HSL],
                                hst_t[:, k, r * 128 : (r + 1) * 128],
                                qk_sb[:, k, :],
                                start=(k == 0),
                                stop=(k == NKS - 1),
                            )
                        nc.scalar.activation(
                            out=pt_sb[:, rt, :],
                            in_=s_ps[:, 0:HSL],
                            func=mybir.ActivationFunctionType.Exp,
                            bias=mb_sb[:, rt : rt + 1],
                            scale=1.0,
                        )

                # den = 1^T p  (one ones-column stationary chain)
                den_ps = dps.tile([128, 512], F32)
                for rt in range(NRT):
                    nc.tensor.matmul(
                        den_ps[0:1, 0:HSL],
                        ones_sb,
                        pt_sb[:, rt, :],
                        start=(rt == 0),
                        stop=(rt == NRT - 1),
                    )
                nc.scalar.copy(out=den_sb, in_=den_ps[0:1, 0:HSL])
                nc.sync.dma_start(out=denT, in_=den_sb)

                # ---- phase 2: u = p^T hs from the row-major half ---------
                # stationary row-major hs blocks [128 s, 128 k], moving p
                for blk in range(RBLK):
                    hsr_t = hsrp.tile([128, NRT, CR], BF16, tag="hsr")
                    for q in range(4):
                        h = NRT // 4
                        nc.sync.dma_start(
                            out=hsr_t[:, q * h : (q + 1) * h],
                            in_=hsrT[blk][:, q * h : (q + 1) * h],
                        )
                    u_ps = ups.tile([128, 512], F32, tag="u")
                    for kc in range(CR // 128):
                        for ss in range(NRT):
                            nc.tensor.matmul(
                                u_ps[:, kc * 128 : kc * 128 + HSL],
                                hsr_t[:, ss, kc * 128 : (kc + 1) * 128],
                                pt_sb[:, ss, :],
                                start=(ss == 0),
                                stop=(ss == NRT - 1),
                            )
                    for kc in range(CR // 128):
                        ks = blk * (CR // 128) + kc
                        eng = nc.vector.tensor_copy if kc % 2 else nc.scalar.copy
                        eng(
                            out=u_sb[:, ks, :],
                            in_=u_ps[:, kc * 128 : kc * 128 + HSL],
                        )

                # ---- phase 3: z = u @ Wv^T (aggregated, tiny) ------------
                z_ps = zps.tile([HSL, BD], F32)
                for k in range(NKS):
                    nc.tensor.matmul(
                        z_ps,
                        u_sb[:, k, :],
                        wv_sb[:, k, :],
                        start=(k == 0),
                        stop=(k == NKS - 1),
                    )
                nc.scalar.copy(out=z_sb, in_=z_ps)
                nc.sync.dma_start(out=zT, in_=z_sb)

    nc.compile()
    return nc


def _get_module():
    if "m" not in _cache:
        _cache["m"] = _build_module()
    return _cache["m"]


def _prep_in_maps(hs, mask, ms, Wq, Wk, Wv, Wo):
    """Shard the full inputs into 8 per-core input maps (host-side)."""
    Q = (ms @ Wq.T).astype(np.float32)  # [SLOTS, BD]
    # qk[h*8+n, :] = (Q_h[n, :] @ Wk_h) / sqrt(HD)
    qk = np.empty((HSL, HID), np.float32)
    for h in range(HEADS):
        qk[h * SLOTS : (h + 1) * SLOTS] = (
            Q[:, h * HD : (h + 1) * HD] @ Wk[h * HD : (h + 1) * HD, :]
        ) * np.float32(1.0 / np.sqrt(HD))
    # [HID, HSL] -> [128 ki, NKS, HSL]
    qkc = np.ascontiguousarray(
        qk.T.reshape(NKS, 128, HSL).transpose(1, 0, 2).astype(npbf16)
    )
    wvc = np.ascontiguousarray(
        Wv.T.reshape(NKS, 128, BD).transpose(1, 0, 2).astype(npbf16)
    )

    in_maps = []
    for c in range(N_CORES):
        b, half = c // HALVES, c % HALVES
        rows = slice(half * SL, (half + 1) * SL)
        hsh = hs[b][rows]  # [SL, HID] f32
        # hsT half: [HID, SL] -> [TBLK, 128 ki, NKS, CT]
        hst = (
            hsh.T.reshape(NKS, 128, TBLK, SL // TBLK)
            .transpose(2, 1, 0, 3)
        )
        # row-major half: [SL, HID] -> [RBLK, 128 si, NRT, CR]
        hsr = (
            hsh.reshape(NRT, 128, RBLK, HID // RBLK)
            .transpose(2, 1, 0, 3)
        )
        mb = (
            np.where(mask[b][rows] == 0, np.float32(MASK_NEG), np.float32(0.0))
            .astype(np.float32)
            .reshape(NRT, 128)
            .T
        )
        in_maps.append(
            {
                "hstT": np.ascontiguousarray(hst.astype(npbf16)),
                "hsrT": np.ascontiguousarray(hsr.astype(npbf16)),
                "qkT": qkc,
                "wvT": wvc,
                "mbT": np.ascontiguousarray(mb),
            }
        )
    return in_maps


def kernel(hidden_states, attention_mask, memory_slots, Wq, Wk, Wv, Wo):
    global LAST_RESULT
    hs = np.asarray(hidden_states, dtype=np.float32)
    mask = np.asarray(attention_mask)
    ms = np.asarray(memory_slots, dtype=np.float32)
    Wq = np.asarray(Wq, dtype=np.float32)
    Wk = np.asarray(Wk, dtype=np.float32)
    Wv = np.asarray(Wv, dtype=np.float32)
    Wo = np.asarray(Wo, dtype=np.float32)

    nc = _get_module()
    in_maps = _prep_in_maps(hs, mask, ms, Wq, Wk, Wv, Wo)

    kwargs = {}
    if TRACE:
        kwargs = {"trace": True}
        if TRACE_CORES is not None:
            kwargs["trace_cores"] = TRACE_CORES
    res = run_bass_kernel_spmd(nc, in_maps, core_ids=list(range(N_CORES)), **kwargs)
    LAST_RESULT = res

    # host combine: sum the two halves' linear partials, normalize per
    # (head, slot), apply the tiny o_proj (67 MFLOP)
    WoH = Wo.reshape(HID, HEADS, HD)  # [out, h, d]
    y = np.empty((B, SLOTS, HID), np.float32)
    for b in range(B):
        z = res.results[2 * b]["zT"] + res.results[2 * b + 1]["zT"]  # [64, 512]
        den = (
            res.results[2 * b]["denT"] + res.results[2 * b + 1]["denT"]
        ).reshape(HEADS, SLOTS)
        o = z.reshape(HEADS, SLOTS, BD)  # [h, n, bd]
        oh = np.empty((SLOTS, HEADS, HD), np.float32)
        for h in range(HEADS):
            oh[:, h, :] = o[h, :, h * HD : (h + 1) * HD] / den[h][:, None]
        y[b] = np.einsum("nhd,ohd->no", oh, WoH)
    return np.ascontiguousarray(y.astype(np.float32))


# revision 45
# speedup vs baseline: 1.7963x; 1.2490x over previous
"""Trainium2 Bass kernel: memory-slot cross-attention (nn_LocalConstructorMulti).

Reference computation (per batch b):
    Q  = memory_slots @ Wq.T                      [slots, BD]    (shared over b)
    K  = hs_b @ Wk.T                              [S, BD]
    V  = hs_b @ Wv.T                              [S, BD]
    s  = (Q_h . K_h) / sqrt(HD)  + mask           [heads, slots, S]
    p  = softmax(s, axis=S)
    o  = p @ V_h                                  [heads, slots, HD]
    y  = concat_h(o) @ Wo.T                       [slots, HID]

NEITHER PROJECTION IS EVER MATERIALIZED.  Both big GEMMs fold away by
associativity around the tiny slot dimension (8 slots x 8 heads = 64):

    scores = Q.(Wk hs^T) = (Q @ Wk) . hs^T        qk: [64, HID], host, 16 MF
    o_num  = p^T (hs Wv^T) = (p^T hs) Wv^T        u:  [HID, 64] on device

That replaces 2 x 4.3 GMAC/core of K/V projections with 2 x 0.54 GMAC of
skinny contractions against hs -- the kernel becomes DMA-bound at the
~332 GB/s HBM stream (cost-model timeline: v1 413.7us -> 111.5us).

Sharding: 8 cores = 4 batches x 2 sequence-halves.  Each core holds its
2048-row half in BOTH orientations (16 MB hsT for the k-contracted score
pass, 16 MB row-major for the s-contracted u pass -- same 32 MB/core as
shipping hs twice was) and computes, for all 8 heads:

    phase 1: s = qk . hsT_half   (stationary hsT blocks, moving qk [128,64];
             mask fused as per-partition Exp bias -> p, all local)
    phase 2: u = p^T hs_half     (stationary row-major hs blocks, moving p;
             den = 1^T p via a ones-column stationary)
    phase 3: z = u @ Wv^T        (32 mov-512 matmuls on the aggregated u)

The host sums the two halves' linear partials (z, den), normalizes, and
applies the 67 MFLOP o_proj (0.05% of the model's FLOPs).  The exp
nonlinearity is the only thing pinning p to the device between the passes;
everything that touches the 256 MB hs tensor stays on-device.

Layout notes: both hs orientations are host-preshuffled into
[blocks, 128 part, subtiles, 512] so every DMA has multi-KiB contiguous
per-partition lines and chains start on pair-group chunk arrival; every
PSUM accumulator owns a full bank or shares one only with other PE-W
accumulation groups (PE-W + DVE/ACT-R same-bank erratum); the score pass
streams first so exp'd p tiles are ready exactly when the u pass needs
them; wv/qk/mb ride outside the hs stream.
"""

import sys

if "/opt/trn_rl_repo" not in sys.path:
    sys.path.insert(0, "/opt/trn_rl_repo")

import ml_dtypes
import numpy as np

import concourse.bass as bass  # noqa: F401  (AP helpers)
import concourse.mybir as mybir
import concourse.tile as tile
from concourse import bacc
from concourse.bass_utils import run_bass_kernel_spmd

BF16 = mybir.dt.bfloat16
FP8 = mybir.dt.float8e4
F32 = mybir.dt.float32
npbf16 = ml_dtypes.bfloat16
npfp8 = ml_dtypes.float8_e4m3

B, S, HID = 4, 4096, 4096
SLOTS, HEADS, BD = 8, 8, 512
HD = BD // HEADS  # 64
N_CORES = 8
HALVES = N_CORES // B  # sequence halves per batch
SL = S // HALVES  # 2048 local rows
HSL = HEADS * SLOTS  # 64 head-slot columns
MASK_NEG = -30000.0

NKS = HID // 128  # 32 k-subtiles
NRT = SL // 128  # 16 local row tiles
TBLK = 4  # hsT column blocks (512 rows each)
RBLK = 8  # row-major k blocks (512 k-cols each)

# test.py can flip this to capture an NTFF profile; harness never touches it.
TRACE = False
TRACE_CORES = None
LAST_RESULT = None

_cache = {}


def _build_module():
    """Emit + compile the single-core Bass module (same NEFF on all cores)."""
    nc = bacc.Bacc("TRN2", target_bir_lowering=False, debug=False, num_devices=N_CORES)

    hstT = nc.dram_tensor("hstT", [TBLK, 128, NKS, SL // TBLK], FP8, kind="ExternalInput").ap()
    hsrT = nc.dram_tensor("hsrT", [RBLK, 128, NRT, HID // RBLK], BF16, kind="ExternalInput").ap()
    qkT = nc.dram_tensor("qkT", [128, NKS, HSL], BF16, kind="ExternalInput").ap()
    wvT = nc.dram_tensor("wvT", [128, NKS, BD], BF16, kind="ExternalInput").ap()
    mbT = nc.dram_tensor("mbT", [128, NRT], F32, kind="ExternalInput").ap()
    zT = nc.dram_tensor("zT", [HSL, BD], F32, kind="ExternalOutput").ap()
    denT = nc.dram_tensor("denT", [1, HSL], F32, kind="ExternalOutput").ap()

    CT = SL // TBLK  # 512 score columns per hsT block
    CR = HID // RBLK  # 512 k columns per row-major block

    with tile.TileContext(nc) as tc:
        with (
            tc.tile_pool(name="consts", bufs=1) as consts,
            tc.tile_pool(name="hstp", bufs=2) as hstp,
            tc.tile_pool(name="hsrp", bufs=2) as hsrp,
        ):
            # ---- resident operands ---------------------------------------
            qk_sb = consts.tile([128, NKS, HSL], BF16)
            wv_sb = consts.tile([128, NKS, BD], BF16)
            mb_sb = consts.tile([128, NRT], F32)
            ones_sb = consts.tile([128, 1], BF16)
            nc.vector.memset(ones_sb, 1.0)

            # ---- persistent intermediates --------------------------------
            pt_sb = consts.tile([128, NRT, HSL], BF16)  # exp(scores)
            u_sb = consts.tile([128, NKS, HSL], BF16)  # u = p^T hs
            den_sb = consts.tile([1, HSL], F32)
            z_sb = consts.tile([HSL, BD], F32)

            with (
                tc.tile_pool(name="sps", bufs=2, space="PSUM") as sps,
                tc.tile_pool(name="ups", bufs=2, space="PSUM") as ups,
                tc.tile_pool(name="dps", bufs=1, space="PSUM") as dps,
                tc.tile_pool(name="zps", bufs=1, space="PSUM") as zps,
            ):
                # ---- phase 1: scores + exp from the k-partitioned half ---
                for blk in range(TBLK):
                    hst_t = hstp.tile([128, NKS, CT], FP8, tag="hst")
                    for q in range(8):
                        h = NKS // 8
                        if blk == 0 and q < 4:
                            # qk/mb ride with the first chunks; they are
                            # tiny and needed by the first chains
                            if q == 0:
                                nc.sync.dma_start(out=qk_sb, in_=qkT)
                                nc.sync.dma_start(out=mb_sb, in_=mbT)
                        nc.sync.dma_start(
                            out=hst_t[:, q * h : (q + 1) * h],
                            in_=hstT[blk][:, q * h : (q + 1) * h],
                        )
                    if blk == TBLK - 1:
                        # wv is first needed by phase 3
                        nc.sync.dma_start(out=wv_sb, in_=wvT)
                    for r in range(CT // 128):
                        rt = blk * (CT // 128) + r
                        s_ps = sps.tile([128, 512], F32, tag="s")
                        for k in range(NKS):
                            nc.tensor.matmul(
                                s_ps[:, 0:HSL],
                                hst_t[:, k, r * 128 : (r + 1) * 128],
                                qk_sb[:, k, :],
                                start=(k == 0),
                                stop=(k == NKS - 1),
                            )
                        nc.scalar.activation(
                            out=pt_sb[:, rt, :],
                            in_=s_ps[:, 0:HSL],
                            func=mybir.ActivationFunctionType.Exp,
                            bias=mb_sb[:, rt : rt + 1],
                            scale=1.0,
                        )

                # den = 1^T p  (one ones-column stationary chain)
                den_ps = dps.tile([128, 512], F32)
                for rt in range(NRT):
                    nc.tensor.matmul(
                        den_ps[0:1, 0:HSL],
                        ones_sb,
                        pt_sb[:, rt, :],
                        start=(rt == 0),
                        stop=(rt == NRT - 1),
                    )
                nc.scalar.copy(out=den_sb, in_=den_ps[0:1, 0:HSL])
                nc.sync.dma_start(out=denT, in_=den_sb)

                # ---- phase 2: u = p^T hs from the row-major half ---------
                # stationary row-major hs blocks [128 s, 128 k], moving p
                for blk in range(RBLK):
                    hsr_t = hsrp.tile([128, NRT, CR], BF16, tag="hsr")
                    for q in range(4):
                        h = NRT // 4
                        nc.sync.dma_start(
                            out=hsr_t[:, q * h : (q + 1) * h],
                            in_=hsrT[blk][:, q * h : (q + 1) * h],
                        )
                    u_ps = ups.tile([128, 512], F32, tag="u")
                    for kc in range(CR // 128):
                        for ss in range(NRT):
                            nc.tensor.matmul(
                                u_ps[:, kc * 128 : kc * 128 + HSL],
                                hsr_t[:, ss, kc * 128 : (kc + 1) * 128],
                                pt_sb[:, ss, :],
                                start=(ss == 0),
                                stop=(ss == NRT - 1),
                            )
                    for kc in range(CR // 128):
                        ks = blk * (CR // 128) + kc
                        eng = nc.vector.tensor_copy if kc % 2 else nc.scalar.copy
                        eng(
                            out=u_sb[:, ks, :],
                            in_=u_ps[:, kc * 128 : kc * 128 + HSL],
                        )

                # ---- phase 3: z = u @ Wv^T (aggregated, tiny) ------------
                z_ps = zps.tile([HSL, BD], F32)
                for k in range(NKS):
                    nc.tensor.matmul(
                        z_ps,
                        u_sb[:, k, :],
                        wv_sb[:, k, :],
                        start=(k == 0),
                        stop=(k == NKS - 1),
                    )
                nc.scalar.copy(out=z_sb, in_=z_ps)
                nc.sync.dma_start(out=zT, in_=z_sb)

    nc.compile()
    return nc


def _get_module():
    if "m" not in _cache:
        _cache["m"] = _build_module()
    return _cache["m"]


def _prep_in_maps(hs, mask, ms, Wq, Wk, Wv, Wo):
    """Shard the full inputs into 8 per-core input maps (host-side)."""
    Q = (ms @ Wq.T).astype(np.float32)  # [SLOTS, BD]
    # qk[h*8+n, :] = (Q_h[n, :] @ Wk_h) / sqrt(HD)
    qk = np.empty((HSL, HID), np.float32)
    for h in range(HEADS):
        qk[h * SLOTS : (h + 1) * SLOTS] = (
            Q[:, h * HD : (h + 1) * HD] @ Wk[h * HD : (h + 1) * HD, :]
        ) * np.float32(1.0 / np.sqrt(HD))
    # [HID, HSL] -> [128 ki, NKS, HSL]
    qkc = np.ascontiguousarray(
        qk.T.reshape(NKS, 128, HSL).transpose(1, 0, 2).astype(npbf16)
    )
    wvc = np.ascontiguousarray(
        Wv.T.reshape(NKS, 128, BD).transpose(1, 0, 2).astype(npbf16)
    )

    in_maps = []
    for c in range(N_CORES):
        b, half = c // HALVES, c % HALVES
        rows = slice(half * SL, (half + 1) * SL)
        hsh = hs[b][rows]  # [SL, HID] f32
        # hsT half: [HID, SL] -> [TBLK, 128 ki, NKS, CT]
        hst = (
            hsh.T.reshape(NKS, 128, TBLK, SL // TBLK)
            .transpose(2, 1, 0, 3)
        )
        # row-major half: [SL, HID] -> [RBLK, 128 si, NRT, CR]
        hsr = (
            hsh.reshape(NRT, 128, RBLK, HID // RBLK)
            .transpose(2, 1, 0, 3)
        )
        mb = (
            np.where(mask[b][rows] == 0, np.float32(MASK_NEG), np.float32(0.0))
            .astype(np.float32)
            .reshape(NRT, 128)
            .T
        )
        in_maps.append(
            {
                "hstT": np.ascontiguousarray(hst.astype(npfp8)),
                "hsrT": np.ascontiguousarray(hsr.astype(npbf16)),
                "qkT": qkc,
                "wvT": wvc,
                "mbT": np.ascontiguousarray(mb),
            }
        )
    return in_maps


def kernel(hidden_states, attention_mask, memory_slots, Wq, Wk, Wv, Wo):
    global LAST_RESULT
    hs = np.asarray(hidden_states, dtype=np.float32)
    mask = np.asarray(attention_mask)
    ms = np.asarray(memory_slots, dtype=np.float32)
    Wq = np.asarray(Wq, dtype=np.float32)
    Wk = np.asarray(Wk, dtype=np.float32)
    Wv = np.asarray(Wv, dtype=np.float32)
    Wo = np.asarray(Wo, dtype=np.float32)

    nc = _get_module()
    in_maps = _prep_in_maps(hs, mask, ms, Wq, Wk, Wv, Wo)

    kwargs = {}
    if TRACE:
        kwargs = {"trace": True}
        if TRACE_CORES is not None:
            kwargs["trace_cores"] = TRACE_CORES
    res = run_bass_kernel_spmd(nc, in_maps, core_ids=list(range(N_CORES)), **kwargs)
    LAST_RESULT = res

    # host combine: sum the two halves' linear partials, normalize per
    # (head, slot), apply the tiny o_proj (67 MFLOP)
    WoH = Wo.reshape(HID, HEADS, HD)  # [out, h, d]
    y = np.empty((B, SLOTS, HID), np.float32)
    for b in range(B):
        z = res.results[2 * b]["zT"] + res.results[2 * b + 1]["zT"]  # [64, 512]
        den = (
            res.results[2 * b]["denT"] + res.results[2 * b + 1]["denT"]
        ).reshape(HEADS, SLOTS)
        o = z.reshape(HEADS, SLOTS, BD)  # [h, n, bd]
        oh = np.empty((SLOTS, HEADS, HD), np.float32)
        for h in range(HEADS):
            oh[:, h, :] = o[h, :, h * HD : (h + 1) * HD] / den[h][:, None]
        y[b] = np.einsum("nhd,ohd->no", oh, WoH)
    return np.ascontiguousarray(y.astype(np.float32))
